# revision 29
# baseline (speedup 1.0000x reference)
"""Trainium2 Bass kernel: sparse AE encoder (L1 fan-in-1 -> relu/BN -> L2 block-diag
4x4 -> relu/BN -> L3 sparse 256-nnz/TF -> BN), SPMD over 8 NeuronCores.

Sharding: gene/hidden axis across cores (BN1/BN2 local: every core holds all 256
batch rows of its features). Layer 1 is matmul-free: x is host-replicated to
hidden-row layout and the ScalarEngine applies relu(w1*x + b1) with per-partition
scale/bias. Layer 2 runs on the TensorEngine with block-diagonal [128,128] bf16
stationaries, layer 3 as a dense matmul against the host-densified W3 shard
(bf16, z[b,t] in four N=512 PSUM accumulators). BN2's mean shift and b3 are
dropped (per-TF constants cancel in BN3), so h2 is scale-only.

v2 schedule: phases A (L1+BN1) and B (L2+BN2+L3) are software-pipelined in
groups of 4 tiles — A leads B by two groups, and each group's L3 matmuls are
deferred until after the next group's L2 matmuls so the TensorEngine never
waits on BN2 normalization. Input DMA is issued in compute order (x first,
then w2/w3 per group). The cross-core reduction of partial z uses AllToAll
(pure-copy mesh exchange, ~2-3x faster than ReduceScatter's CCE path) followed
by a local f32 tree-reduce on the VectorEngine; each core then applies BN3 to
its own 128 TF rows and emits a [128, 256] outT shard.
"""

import numpy as np
import ml_dtypes

import concourse.bacc as bacc
import concourse.bass as bass
import concourse.tile as tile
import concourse.mybir as mybir
from concourse import bass_utils
from concourse.masks import make_identity

N_GENES = 8192
WM = 4
HID = N_GENES * WM          # 32768
N_TF = 1024
B = 256
EPS = 1e-5

NCORES = 8
GSH = N_GENES // NCORES     # 1024 genes / core
HSH = HID // NCORES         # 4096 hidden rows / core
P = 128
NT = HSH // P               # 32 hidden tiles / core
GB = 4                      # pipeline group size (tiles)
NG = NT // GB               # 8 groups

BF16 = ml_dtypes.bfloat16
F32 = mybir.dt.float32
F16 = mybir.dt.float16
BF = mybir.dt.bfloat16
I32 = mybir.dt.int32
AF = mybir.ActivationFunctionType
OP = mybir.AluOpType

TRACE = False
LAST_RESULT = None

_cache = {}


def _build_graph():
    nc = bacc.Bacc("TRN2", target_bir_lowering=False, debug=False, num_devices=NCORES)

    xrd = nc.dram_tensor("xrd", [P, NT * B], BF, kind="ExternalInput").ap()
    w1d = nc.dram_tensor("w1d", [P, NT], F32, kind="ExternalInput").ap()
    w2d = nc.dram_tensor("w2d", [P, NT * P], BF, kind="ExternalInput").ap()
    w3d = nc.dram_tensor("w3d", [P, NT * N_TF], BF, kind="ExternalInput").ap()
    b1d = nc.dram_tensor("b1d", [P, NT], F32, kind="ExternalInput").ap()
    b2d = nc.dram_tensor("b2d", [P, NT], F32, kind="ExternalInput").ap()
    outT = nc.dram_tensor("outT", [P, B], F32, kind="ExternalOutput").ap()

    from contextlib import ExitStack
    with tile.TileContext(nc) as tc, ExitStack() as ctx:
        cpool = ctx.enter_context(tc.tile_pool(name="const", bufs=1))
        wpool = ctx.enter_context(tc.tile_pool(name="wts", bufs=1))
        apool = ctx.enter_context(tc.tile_pool(name="acts", bufs=1))
        spool = ctx.enter_context(tc.tile_pool(name="stats", bufs=4))
        ztpool = ctx.enter_context(tc.tile_pool(name="ztile", bufs=2))
        psAB = ctx.enter_context(tc.tile_pool(name="psAB", bufs=2, space="PSUM"))
        psZp = ctx.enter_context(tc.tile_pool(name="psZ", bufs=1, space="PSUM"))
        psTp = ctx.enter_context(tc.tile_pool(name="psT", bufs=2, space="PSUM"))
        dpool = ctx.enter_context(tc.tile_pool(name="dram", bufs=1, space="DRAM"))

        # ---- warmup collective -------------------------------------------
        # Collective costs here are strongly order-dependent: #1 (the bass
        # prelude barrier) pays ncfw boot + a ~35us cold pass, #2 pays a
        # ~10us scheduling gap + semi-cold mesh, #3+ run warm.  A tiny dummy
        # AllToAll occupies slot #2 so the real one runs warm.  Its input is
        # deliberately never written (output is ignored) so the trigger has
        # no DMA dependency and the GpSimd queue never blocks on it.
        dum_in = dpool.tile([NCORES, NCORES], BF, name="dum_in")
        dum_out = dpool.tile([NCORES, NCORES], BF, name="dum_out")
        nc.gpsimd.collective_compute(
            "AllToAll", OP.bypass, replica_groups=[list(range(NCORES))],
            ins=[dum_in.opt()], outs=[dum_out.opt()])

        # ---- static loads, issued in compute-consumption order -----------
        b1t = cpool.tile([P, NT], F32, name="b1t")
        nc.sync.dma_start(b1t[:], b1d[:])
        w1t = cpool.tile([P, NT], F32, name="w1t")
        nc.sync.dma_start(w1t[:], w1d[:])
        b2t = cpool.tile([P, NT], F32, name="b2t")
        nc.sync.dma_start(b2t[:], b2d[:])

        xrep = wpool.tile([P, NT * B], BF, name="xrep")
        w2s = wpool.tile([P, NT * P], BF, name="w2s")
        w3s = wpool.tile([P, NT * N_TF], BF, name="w3s")

        def load_x(c):     # 4 chunks of 8 tiles each (512KB)
            xcw = 8 * B
            nc.sync.dma_start(xrep[:, c * xcw:(c + 1) * xcw],
                              xrd[:, c * xcw:(c + 1) * xcw])

        def load_w2(c):    # 4 chunks of 8 tiles each
            cw = 8 * P
            nc.sync.dma_start(w2s[:, c * cw:(c + 1) * cw],
                              w2d[:, c * cw:(c + 1) * cw])

        def load_w3(c):    # 8 chunks of 4 tiles each (1MB)
            cw = GB * N_TF
            nc.sync.dma_start(w3s[:, c * cw:(c + 1) * cw],
                              w3d[:, c * cw:(c + 1) * cw])

        # Interleave x chunks behind the heavy w3 loads: the Tile scheduler
        # orders each engine queue off its simulated timeline, and
        # front-loading all of x makes it sort every phase-A relu ahead of
        # the BN sqrts, stalling the whole pipeline.
        load_x(0); load_w2(0); load_x(1)
        load_w3(0); load_w3(1)
        load_x(2); load_w2(1); load_w3(2); load_w3(3)
        load_x(3); load_w2(2); load_w3(4); load_w3(5)
        load_w2(3); load_w3(6); load_w3(7)

        idt = cpool.tile([P, P], F16, name="idt")
        make_identity(nc, idt[:])

        # constants for the gpsimd fast-inverse-sqrt (no scalar-engine sqrt:
        # keeping Sqrt off the Scalar queue keeps it homogeneous relus, which
        # the Tile scheduler cannot reorder into a pipeline stall)
        onest = cpool.tile([P, GB], I32, name="onest")
        nc.gpsimd.memset(onest[:], 1)
        magict = cpool.tile([P, GB], I32, name="magict")
        nc.gpsimd.memset(magict[:], 0x5F3759DF)

        def rsqrt_gp(istd, var, w=GB, eng=None):
            """istd = 1/sqrt(var): bit-trick seed (Vector) + 2 Newton steps
            on `eng` (Pool-engine shifts need int64, so seeds stay on DVE)."""
            if eng is None:
                eng = nc.gpsimd
            yi = spool.tile([P, GB], I32, name="yi", tag="yi")
            nc.vector.tensor_tensor(yi[:, :w], var.bitcast(I32), onest[:, :w],
                                    op=OP.arith_shift_right)
            nc.vector.tensor_tensor(yi[:, :w], magict[:, :w], yi[:, :w],
                                    op=OP.subtract)
            y0 = yi[:, :w].bitcast(F32)
            t = spool.tile([P, GB], F32, name="nt", tag="nt")
            y1 = spool.tile([P, GB], F32, name="y1", tag="y1")
            eng.tensor_tensor(t[:, :w], y0, y0, op=OP.mult)
            eng.tensor_tensor(t[:, :w], t[:, :w], var, op=OP.mult)
            eng.tensor_scalar(out=t[:, :w], in0=t[:, :w], scalar1=-0.5,
                              scalar2=1.5, op0=OP.mult, op1=OP.add)
            eng.tensor_tensor(y1[:, :w], y0, t[:, :w], op=OP.mult)
            eng.tensor_tensor(t[:, :w], y1[:, :w], y1[:, :w], op=OP.mult)
            eng.tensor_tensor(t[:, :w], t[:, :w], var, op=OP.mult)
            eng.tensor_scalar(out=t[:, :w], in0=t[:, :w], scalar1=-0.5,
                              scalar2=1.5, op0=OP.mult, op1=OP.add)
            eng.tensor_tensor(istd, y1[:, :w], t[:, :w], op=OP.mult)

        hrA = apool.tile([P, NT * B], BF, name="hrA")
        h1n = apool.tile([P, NT * B], BF, name="h1n")
        hrB = apool.tile([P, NT * B], BF, name="hrB")
        h2n = apool.tile([P, NT * B], BF, name="h2n")
        st1 = apool.tile([P, NT * 6], F32, name="st1")
        st2 = apool.tile([P, NT * 6], F32, name="st2")

        # layer-3 psums, accumulated across all NT tiles
        psZ = [[psZp.tile([P, 512], F32, name=f"psZ{bh}{th}", tag=f"psZ{bh}{th}")
                for th in range(2)] for bh in range(2)]

        def norm_params(st, g0, istd, nm, scale_only, rsq_eng=None):
            """bn_stats 6-tuples (even/odd halves) -> istd (and -mean*istd)."""
            sv = st[:, g0 * 6:(g0 + GB) * 6].rearrange("p (t s) -> p t s", s=6)
            me, mo = sv[:, :, 1], sv[:, :, 4]
            M2e, M2o = sv[:, :, 2], sv[:, :, 5]
            dm = spool.tile([P, GB], F32, name="dm", tag="dm")
            nc.vector.scalar_tensor_tensor(dm[:], in0=me, scalar=1.0, in1=mo,
                                           op0=OP.mult, op1=OP.subtract)
            vq = spool.tile([P, GB], F32, name="vq", tag="vq")
            nc.vector.scalar_tensor_tensor(vq[:], in0=dm[:], scalar=0.25, in1=dm[:],
                                           op0=OP.mult, op1=OP.mult)
            var = spool.tile([P, GB], F32, name="var", tag="var")
            nc.vector.scalar_tensor_tensor(var[:], in0=M2e[:, :], scalar=1.0,
                                           in1=M2o[:, :], op0=OP.mult, op1=OP.add)
            nc.vector.tensor_scalar(out=var[:], in0=var[:], scalar1=1.0 / B,
                                    scalar2=EPS, op0=OP.mult, op1=OP.add)
            nc.vector.tensor_tensor(var[:], var[:], vq[:], op=OP.add)
            rsqrt_gp(istd[:], var[:], eng=rsq_eng)
            if not scale_only:
                ms = spool.tile([P, GB], F32, name="ms", tag="ms")
                nc.vector.scalar_tensor_tensor(ms[:], in0=me, scalar=1.0, in1=mo,
                                               op0=OP.mult, op1=OP.add)
                nc.vector.tensor_scalar(out=ms[:], in0=ms[:], scalar1=-0.5,
                                        scalar2=None, op0=OP.mult)
                nc.gpsimd.tensor_tensor(nm[:], ms[:], istd[:], op=OP.mult)

        def emitA(g):
            """L1 relu (Scalar) + BN1 stats (Vector) + normalize (GpSimd)."""
            for t in range(g * GB, (g + 1) * GB):
                hrt = hrA[:, t * B:(t + 1) * B]
                nc.scalar.activation(hrt, xrep[:, t * B:(t + 1) * B], AF.Relu,
                                     bias=b1t[:, t:t + 1], scale=w1t[:, t:t + 1])
                nc.vector.bn_stats(st1[:, t * 6:(t + 1) * 6], hrt)
            istd = spool.tile([P, GB], F32, name="istdA", tag="istdA")
            nm = spool.tile([P, GB], F32, name="nmA", tag="nmA")
            norm_params(st1, g * GB, istd, nm, scale_only=False)
            for t in range(g * GB, (g + 1) * GB):
                j = t - g * GB
                nc.gpsimd.tensor_scalar(out=h1n[:, t * B:(t + 1) * B],
                                        in0=hrA[:, t * B:(t + 1) * B],
                                        scalar1=istd[:, j:j + 1],
                                        scalar2=nm[:, j:j + 1],
                                        op0=OP.mult, op1=OP.add)

        def emitB_front(g):
            """L2 matmul (PE) + relu (Scalar) + BN2 stats (Vector)."""
            ps2 = None
            for t in range(g * GB, (g + 1) * GB):
                if t % 2 == 0:  # one PSUM bank holds two tiles' L2 outputs
                    ps2 = psAB.tile([P, 2 * B], F32, name="psL", tag="psL")
                ps = ps2[:, (t % 2) * B:(t % 2 + 1) * B]
                nc.tensor.matmul(ps, lhsT=w2s[:, t * P:(t + 1) * P],
                                 rhs=h1n[:, t * B:(t + 1) * B],
                                 start=True, stop=True, skip_group_check=True)
                hrt = hrB[:, t * B:(t + 1) * B]
                nc.scalar.activation(hrt, ps, AF.Relu, bias=b2t[:, t:t + 1])
                nc.vector.bn_stats(st2[:, t * 6:(t + 1) * 6], hrt)

        def emitB_norm(g):
            """BN2 scale-only normalize; mean shift cancels in BN3.  The istd
            Newton steps stay on Vector (in-order with the stats), the
            normalizes go to GpSimd; L3 is deferred two groups so this
            cross-engine chain is never on the TensorEngine critical path."""
            istd = spool.tile([P, GB], F32, name="istdB", tag="istdB")
            norm_params(st2, g * GB, istd, None, scale_only=True,
                        rsq_eng=nc.vector)
            for t in range(g * GB, (g + 1) * GB):
                j = t - g * GB
                nc.gpsimd.tensor_scalar(out=h2n[:, t * B:(t + 1) * B],
                                        in0=hrB[:, t * B:(t + 1) * B],
                                        scalar1=istd[:, j:j + 1],
                                        scalar2=None, op0=OP.mult)

        def emitL3(g):
            """z accumulation: deferred one group so PE never waits on BN2."""
            for t in range(g * GB, (g + 1) * GB):
                for bh in range(2):
                    for th in range(2):
                        nc.tensor.matmul(
                            psZ[bh][th][:],
                            lhsT=h2n[:, t * B + bh * P: t * B + (bh + 1) * P],
                            rhs=w3s[:, t * N_TF + th * 512: t * N_TF + (th + 1) * 512],
                            start=(t == 0), stop=(t == NT - 1),
                            skip_group_check=True)

        emitA(0)
        emitA(1)
        pend = []
        for g in range(NG):
            emitB_front(g)
            emitB_norm(g)   # before emitA so L3 deps clear Vector first
            if len(pend) == 2:
                emitL3(pend.pop(0))
            if g + 2 < NG:
                emitA(g + 2)
            pend.append(g)
        for g in pend:
            emitL3(g)

        # ---- drain z, transpose to z^T, AllToAll, local reduce, BN3 ------
        zpart = apool.tile([P, 2 * N_TF], F16, name="zpart")
        for th in range(2):  # split across Scalar/Vector to halve drain time
            nc.scalar.activation(
                zpart[:, th * 512:(th + 1) * 512], psZ[0][th][:], AF.Copy)
            nc.vector.tensor_copy(
                zpart[:, N_TF + th * 512: N_TF + (th + 1) * 512], psZ[1][th][:])

        zinT = dpool.tile([N_TF, B], F16, name="zinT")
        for q in range(2):  # two DMA batches of 4 transposed tiles each
            zT4 = ztpool.tile([P, 4 * B], F16, name="zT4", tag="zT4")
            for k in range(4):
                tt = q * 4 + k
                for bh in range(2):
                    pst = psTp.tile([P, P], F16, name="pst", tag="pst")
                    nc.tensor.transpose(
                        pst[:], in_=zpart[:, bh * N_TF + tt * P: bh * N_TF + (tt + 1) * P],
                        identity=idt[:])
                    nc.vector.tensor_copy(zT4[:, k * B + bh * P: k * B + (bh + 1) * P],
                                          pst[:])
            nc.gpsimd.dma_start(
                zinT[q * 512:(q + 1) * 512, :].rearrange("(k p) b -> p k b", p=P),
                zT4[:].rearrange("p (k b) -> p k b", k=4))

        # AllToAll: rank r sends TF-block j to rank j; receives its own 128
        # TF rows' partials from every rank, then reduces locally on Vector.
        za = dpool.tile([N_TF, B], F16, name="za")
        nc.gpsimd.collective_compute(
            "AllToAll", OP.bypass, replica_groups=[list(range(NCORES))],
            ins=[zinT.opt()], outs=[za.opt()])

        # Load the 8 received partials as 4 pair-chunks so the first adds
        # overlap the remaining loads, then finish the tree in f32.
        zsum = ztpool.tile([P, NCORES * B], F16, name="zsum", tag="zsum")
        t1 = ztpool.tile([P, 4 * B], F32, name="t1", tag="t1")
        for k in range(4):
            nc.sync.dma_start(zsum[:, k * B:(k + 1) * B],
                              za[k * P:(k + 1) * P, :])
            nc.sync.dma_start(zsum[:, (4 + k) * B:(5 + k) * B],
                              za[(4 + k) * P:(5 + k) * P, :])
            nc.vector.tensor_tensor(t1[:, k * B:(k + 1) * B],
                                    zsum[:, k * B:(k + 1) * B],
                                    zsum[:, (4 + k) * B:(5 + k) * B], op=OP.add)
        t2 = ztpool.tile([P, 2 * B], F32, name="t2", tag="t2")
        nc.vector.tensor_tensor(t2[:], t1[:, 0:2 * B], t1[:, 2 * B:4 * B],
                                op=OP.add)
        zs = ztpool.tile([P, B], F32, name="zs", tag="zs")
        nc.vector.tensor_tensor(zs[:], t2[:, 0:B], t2[:, B:2 * B], op=OP.add)

        st6 = spool.tile([P, 6], F32, name="st6", tag="st6")
        nc.vector.bn_stats(st6[:], zs[:])
        mv3 = spool.tile([P, 2], F32, name="mv3", tag="mv3")
        nc.vector.bn_aggr(mv3[:], st6[:])
        var3 = spool.tile([P, 1], F32, name="var3", tag="var3")
        nc.vector.tensor_scalar(out=var3[:], in0=mv3[:, 1:2], scalar1=1.0,
                                scalar2=EPS, op0=OP.mult, op1=OP.add)
        istd3 = spool.tile([P, 1], F32, name="istd3", tag="istd3")
        rsqrt_gp(istd3[:], var3[:], w=1)
        nm3 = spool.tile([P, 1], F32, name="nm3", tag="nm3")
        nc.vector.scalar_tensor_tensor(nm3[:], in0=mv3[:, 0:1], scalar=-1.0,
                                       in1=istd3[:], op0=OP.mult, op1=OP.mult)
        ofin = ztpool.tile([P, B], F32, name="ofin", tag="ofin")
        nc.vector.tensor_scalar(out=ofin[:], in0=zs[:], scalar1=istd3[:],
                                scalar2=nm3[:], op0=OP.mult, op1=OP.add)
        nc.sync.dma_start(outT[:], ofin[:])

    nc.compile()
    return nc


def _pack_inputs(features, w1, b1, w2, b2, w3, b3,
                 rows1, cols1, rows2, cols2, rows3, cols3):
    """Host-side packing into per-core contiguous [128, N] tile layouts."""
    f32 = np.float32
    features = np.asarray(features, f32)
    w1 = np.asarray(w1, f32); b1 = np.asarray(b1, f32)
    w2 = np.asarray(w2, f32); b2 = np.asarray(b2, f32)
    w3 = np.asarray(w3, f32)
    rows1 = np.asarray(rows1); cols1 = np.asarray(cols1)
    rows2 = np.asarray(rows2); cols2 = np.asarray(cols2)
    rows3 = np.asarray(rows3); cols3 = np.asarray(cols3)

    w1r = np.empty(HID, f32); w1r[rows1] = w1
    c1r = np.empty(HID, np.int64); c1r[rows1] = cols1

    order2 = np.argsort(rows2, kind="stable")
    r2 = rows2[order2]; c2 = cols2[order2]; v2 = w2[order2]

    W3d = np.zeros((HID, N_TF), f32)
    np.add.at(W3d, (cols3.astype(np.int64), rows3.astype(np.int64)), w3)

    featT = np.ascontiguousarray(features.T)  # [N_GENES, B]
    in_maps = []
    for c in range(NCORES):
        hbase = c * HSH
        # xrd[p, t*B:b] = features[b, gene_of(hid row hbase + t*128 + p)]
        genes = c1r[hbase:hbase + HSH]                      # [HSH]
        xrep = featT[genes].reshape(NT, P, B).transpose(1, 0, 2).reshape(P, NT * B)

        w2t = np.zeros((NT, P, P), f32)
        for t in range(NT):
            R0 = hbase + t * P
            es = slice(WM * R0, WM * (R0 + P))
            np.add.at(w2t[t], (c2[es] - R0, r2[es] - R0), v2[es])

        w3t = W3d[hbase:hbase + HSH].reshape(NT, P, N_TF)

        in_maps.append({
            "xrd": np.ascontiguousarray(xrep).astype(BF16),
            "w1d": np.ascontiguousarray(w1r[hbase:hbase + HSH].reshape(NT, P).T),
            "w2d": np.ascontiguousarray(w2t.transpose(1, 0, 2).reshape(P, NT * P)).astype(BF16),
            "w3d": np.ascontiguousarray(w3t.transpose(1, 0, 2).reshape(P, NT * N_TF)).astype(BF16),
            "b1d": np.ascontiguousarray(b1[hbase:hbase + HSH].reshape(NT, P).T),
            "b2d": np.ascontiguousarray(b2[hbase:hbase + HSH].reshape(NT, P).T),
        })
    return in_maps


def kernel(**inputs) -> np.ndarray:
    global LAST_RESULT
    if "nc" not in _cache:
        _cache["nc"] = _build_graph()
    nc = _cache["nc"]

    in_maps = _pack_inputs(**inputs)
    # b3 is dropped: BN3 subtracts the per-TF batch mean, so a per-TF constant
    # bias cancels exactly.

    res = bass_utils.run_bass_kernel_spmd(
        nc, in_maps, core_ids=list(range(NCORES)), trace=TRACE)
    LAST_RESULT = res

    outT = np.concatenate([res.results[c]["outT"] for c in range(NCORES)], axis=0)
    return np.ascontiguousarray(outT.T.astype(np.float32))
